# revision 24
# baseline (speedup 1.0000x reference)
"""GAT layer kernel for Trainium2, 8 NeuronCores.

Problem: nn_GATLayer (B=4, N=2048, IN_F=256, OUT_F=64, H=8).

Key algebra: softmax over j of (src[b,i,h] + dst[b,j,h]) masked by adj[b,i,j].
src[b,i,h] is constant over j, so it cancels in the softmax:
    out[b,i,(h,f)] = (adj[b,i,:] @ g[b,:, (h,f)]) / (adj[b,i,:] @ e[b,:,h])
with hfeat = x@W (per-head features), dst[j,h] = x[j,:] @ (W . attn_dst)[:,h],
e = exp(dst), g = e * hfeat.  attn_src is mathematically irrelevant.

Sharding: 8 cores = 4 batches x 2 row-halves of i (softmax is over j only,
so row-sharding of i needs no communication).

Numerics/perf design:
  - bf16 projection (host casts x/W, folds attn_dst into the weight).
  - e = exp(dst + ln(1/16)) kept in bf16; the global 1/16 scale cancels in
    the softmax ratio and keeps the fp8 g tensors inside e4m3 range.
  - g = e*hfeat is split into g_hi + g_lo (e4m3 value + e4m3 residual).
    The numerator uses fp8 DoubleRow matmuls (4x MAC rate, 2 j-chunks per
    instruction) on both halves: bf16-level accuracy at 2x bf16 speed.
  - The denominator contracts adjT(fp8) against e(bf16) with plain
    matmuls; all 8 ics' denominators accumulate in ONE psum bank
    ([128,8,8] slices, single bank-zeroing start).
  - Elementwise pipeline per j-chunk: DVE multiply (psum f32 -> bf16),
    ACT cast to g_hi, Pool/DVE subtract for g_lo; spread so no engine
    exceeds the PE pace.
  - PE pre-warm dummy matmuls bridge the tensor-engine p-state ramp;
    input DMAs go through HWDGE (sync) to keep Pool free for subtracts.
"""

import numpy as np
import ml_dtypes

B, N, IN_F, OUT_F, H = 4, 2048, 256, 64, 8
HF = H * OUT_F            # 512 concat features
NCORES = 8
ROWS = B * N // NCORES    # 1024 destination rows per core
P = 128
IC = ROWS // P            # 8 i-chunks per core
JC = N // P               # 16 j-chunks
KC = IN_F // P            # 2 k-chunks
JG = 8                    # adjT DMA groups == DoubleRow j-chunk pairs
WAVE = 4                  # i-chunks in the first (interleaved) wave
NDUMMY = 6                # PE p-state pre-warm matmuls
ESCALE = 0.0625           # global scale on e; cancels in the softmax ratio

BF16 = ml_dtypes.bfloat16
FP8 = ml_dtypes.float8_e4m3

_CACHE = {}


def _bcast_last(ap, n):
    """View ap with an extra innermost broadcast (stride-0) dim of size n."""
    ap2 = ap.unsqueeze(len(ap.shape))
    return ap2.broadcast_to(tuple(ap.shape) + (n,))


def _build():
    import concourse.mybir as mybir
    import concourse.tile as tile
    from concourse import bacc

    f32 = mybir.dt.float32
    bf = mybir.dt.bfloat16
    fp8 = mybir.dt.float8e4
    MULT = mybir.AluOpType.mult
    SUB = mybir.AluOpType.subtract
    DR = mybir.MatmulPerfMode.DoubleRow

    nc = bacc.Bacc(trn_type="TRN2", debug=False, target_bir_lowering=False)

    adjt_d = nc.dram_tensor("adjt", [N, ROWS], fp8, kind="ExternalInput")
    w_d = nc.dram_tensor("w", [P, KC * HF], bf, kind="ExternalInput")
    wdst_d = nc.dram_tensor("wdst", [P, KC * H], bf, kind="ExternalInput")
    xt_d = nc.dram_tensor("xt", [P, 2 * KC * (N // 2)], bf, kind="ExternalInput")
    out_ds = [
        nc.dram_tensor(f"out{q}", [P, HF], bf, kind="ExternalOutput")
        for q in range(IC)
    ]

    with tile.TileContext(nc) as tc:
        with (
            tc.tile_pool(name="setup", bufs=1) as setup,
            tc.tile_pool(name="gpool", bufs=1) as gpool,
            tc.tile_pool(name="scratch", bufs=2) as scr,
            tc.tile_pool(name="adjT", bufs=1) as adjTp,
            tc.tile_pool(name="warm", bufs=1) as warmp,
            tc.tile_pool(name="ps_num", bufs=4, space="PSUM") as psnum,
            tc.tile_pool(name="ps_h", bufs=2, space="PSUM") as psh,
            tc.tile_pool(name="ps_dst", bufs=1, space="PSUM") as psdst,
            tc.tile_pool(name="ps_den", bufs=1, space="PSUM") as psden,
            tc.tile_pool(name="nsbp", bufs=4) as nsbp,
        ):
            # --- PE pre-warm: dummies into pF0; its first real matmul
            # re-starts the accumulation group, so no extra bank is used ---
            pFs = [psnum.tile([P, HF], f32, tag="num", name=f"pF0_{k}")
                   for k in range(WAVE)]
            junk = warmp.tile([P, P], bf)
            nc.vector.memset(junk[:], 0.0)
            ebias = warmp.tile([P, 1], f32)
            nc.vector.memset(ebias[:], float(np.log(ESCALE)))
            junk_rhs = junk[:, 0:P].unsqueeze(1).broadcast_to((P, 4, P))
            for _ in range(NDUMMY):
                nc.tensor.matmul(pFs[0][:], junk[:], junk_rhs,
                                 start=True, stop=True, skip_group_check=True)

            # --- input streams via HWDGE (sync), ordered by consumption;
            # Pool stays free for the g_lo subtracts ---
            w_sb = setup.tile([P, KC, HF], bf)
            wdst_sb = setup.tile([P, KC, H], bf)
            xT_sb = setup.tile([P, 2, KC, N // 2], bf)
            adjT_g = []
            for G in range(JG):
                adjT_g.append(adjTp.tile([P, 2, ROWS], fp8, tag=f"adjt{G}",
                                         name=f"adjt{G}"))

            w_v = w_d.rearrange("p (kc n) -> p kc n", kc=KC)
            xt_v = xt_d.rearrange("p (jh kc j) -> p jh kc j", jh=2, kc=KC)

            def load_slab(s):
                jh, j0 = divmod(s * (N // 4), N // 2)
                nc.sync.dma_start(
                    xT_sb[:, jh, :, j0:j0 + N // 4],
                    xt_v[:, jh, :, j0:j0 + N // 4])

            def load_adjt(G):
                nc.sync.dma_start(
                    adjT_g[G][:],
                    adjt_d[G * 2 * P:(G + 1) * 2 * P, :].rearrange(
                        "(jc jp) i -> jp jc i", jp=P),
                )

            nc.sync.dma_start(
                wdst_sb[:], wdst_d.rearrange("p (kc h) -> p kc h", kc=KC))
            load_slab(0)
            for kc in range(KC):
                nc.sync.dma_start(w_sb[:, kc], w_v[:, kc])
            load_adjt(0)
            load_adjt(1)
            load_slab(1)
            load_adjt(2)
            load_slab(2)
            load_adjt(3)
            load_slab(3)
            for G in range(4, JG):
                load_adjt(G)

            def xT(kc, jc):
                jh, j0 = divmod(jc * P, N // 2)
                return xT_sb[:, jh, kc, j0:j0 + P]

            g_bf = gpool.tile([P, JC, HF], bf)
            g_hi = gpool.tile([P, JC, HF], fp8)
            g_lo = gpool.tile([P, JC, HF], fp8)
            e_sb = gpool.tile([P, JC, H], bf)
            pdst = psdst.tile([P, JC, H], f32)
            pden = psden.tile([P, IC, H], f32)

            def proj(jc):
                # dst first: the exp -> multiply -> cast -> subtract chain
                # hangs off it, so give ACT/DVE/Pool a head start
                for kc in range(KC):
                    nc.tensor.matmul(
                        pdst[:, jc, :], xT(kc, jc), wdst_sb[:, kc, :],
                        start=(jc == 0 and kc == 0), stop=(kc == KC - 1),
                        skip_group_check=True,
                    )
                ph = psh.tile([P, HF], f32, tag="hfeat")
                for kc in range(KC):
                    nc.tensor.matmul(
                        ph[:], xT(kc, jc), w_sb[:, kc, :],
                        start=(kc == 0), stop=(kc == KC - 1),
                    )
                e_cols = e_sb[:, jc, :]
                nc.scalar.activation(
                    e_cols, pdst[:, jc, :], mybir.ActivationFunctionType.Exp,
                    bias=ebias[:],
                )
                gb = g_bf[:, jc, :]
                b3 = gb.rearrange("p (h f) -> p h f", h=H)
                h3 = ph[:].rearrange("p (h f) -> p h f", h=H)
                e3 = _bcast_last(e_cols, OUT_F)
                nc.vector.tensor_tensor(b3, h3, e3, op=MULT)
                nc.scalar.activation(
                    g_hi[:, jc, :], gb, mybir.ActivationFunctionType.Copy)
                # residual: all-SBUF so Pool may take it (GPSIMD cannot
                # touch PSUM); a quarter goes to DVE for balance
                eng = nc.vector if jc % 4 == 0 else nc.gpsimd
                eng.tensor_tensor(g_lo[:, jc, :], gb, g_hi[:, jc, :], op=SUB)

            def mm_group(pF, ic, c):
                """DoubleRow numerator (hi+lo) and bf16 denominator for
                j-chunk pair c (j-chunks 2c, 2c+1) into i-chunk ic."""
                lhs2 = adjT_g[c][:, :, ic * P:(ic + 1) * P]
                nc.tensor.matmul(
                    pF[:], lhs2, g_hi[:, 2 * c:2 * c + 2, :],
                    start=(c == 0), stop=False, perf_mode=DR,
                    skip_group_check=True,
                )
                nc.tensor.matmul(
                    pF[:], lhs2, g_lo[:, 2 * c:2 * c + 2, :],
                    start=False, stop=(c == JG - 1), perf_mode=DR,
                    skip_group_check=True,
                )
                for t in range(2):
                    jc = 2 * c + t
                    nc.tensor.matmul(
                        pden[:, ic, :], adjT_g[c][:, t, ic * P:(ic + 1) * P],
                        e_sb[:, jc, :],
                        start=(ic == 0 and jc == 0),
                        stop=(jc == JC - 1),
                        skip_group_check=True,
                    )

            def fin_mul(ic, pF, rc_ap, split=False):
                nsb = nsbp.tile([P, HF], bf, tag="nsb", name=f"nsb{ic}")
                n3 = nsb[:].rearrange("p (h f) -> p h f", h=H)
                p3 = pF[:].rearrange("p (h f) -> p h f", h=H)
                r3 = _bcast_last(rc_ap, OUT_F)
                if split:
                    # last output: halve the multiply so the first store
                    # overlaps the second half's multiply
                    for hh in range(2):
                        sl = slice(hh * (H // 2), (hh + 1) * (H // 2))
                        nc.vector.tensor_tensor(
                            n3[:, sl], p3[:, sl], r3[:, sl], op=MULT)
                        nc.sync.dma_start(
                            out_ds[ic][:, hh * (HF // 2):(hh + 1) * (HF // 2)],
                            nsb[:, hh * (HF // 2):(hh + 1) * (HF // 2)])
                else:
                    nc.vector.tensor_tensor(n3, p3, r3, op=MULT)
                    nc.sync.dma_start(out_ds[ic][:, :], nsb[:])

            # --- wave 0: projection interleaved with ics 0-3.  Aggregation
            # runs LAG j-chunks behind projection so the elementwise chain
            # is off the PE critical path (waiting matmuls would clog the
            # depth-4 PE wait queue). ---
            LAG = 4
            for step in range(JC + LAG):
                if step < JC:
                    proj(step)
                ready = step - LAG
                if ready >= 0 and ready % 2 == 1:
                    c = ready // 2
                    for k in range(WAVE):
                        mm_group(pFs[k], k, c)
            # one batched reciprocal for all wave-0 denominators
            rc0 = scr.tile([P, WAVE, H], f32, tag="rc0")
            nc.vector.reciprocal(rc0[:], pden[:, 0:WAVE, :])
            for k in range(WAVE):
                fin_mul(k, pFs[k], rc0[:, k, :])

            # --- wave 1: ics 4-7, ic-major; finalize staggered one ic
            # behind (issued mid-way through the next ic's matmuls) ---
            pF1 = []
            rc1 = scr.tile([P, WAVE, H], f32, tag="rc1")
            for k in range(WAVE):
                ic = WAVE + k
                pF = psnum.tile([P, HF], f32, tag="num", name=f"pF1_{k}")
                pF1.append(pF)
                for c in range(JG):
                    mm_group(pF, ic, c)
                    if c == 3 and k > 0:
                        pic = WAVE + k - 1
                        nc.vector.reciprocal(
                            rc1[:, k - 1, :], pden[:, pic, :])
                        fin_mul(pic, pF1[k - 1], rc1[:, k - 1, :])
            nc.vector.reciprocal(rc1[:, WAVE - 1, :], pden[:, 2 * WAVE - 1, :])
            fin_mul(2 * WAVE - 1, pF1[WAVE - 1], rc1[:, WAVE - 1, :],
                    split=True)

    nc.compile()
    return nc


def _get_nc():
    if "nc" not in _CACHE:
        _CACHE["nc"] = _build()
    return _CACHE["nc"]


def _make_in_maps(x, adj, weight, attn_dst):
    x = np.ascontiguousarray(np.asarray(x), dtype=np.float32)
    adj = np.asarray(adj)
    weight = np.ascontiguousarray(np.asarray(weight), dtype=np.float32)
    attn_dst = np.ascontiguousarray(np.asarray(attn_dst), dtype=np.float32)

    # fold attn_dst into the weight: wdst[k, h] = sum_f W[k, h*64+f]*adst[h, f]
    wdst = (weight.reshape(IN_F, H, OUT_F) * attn_dst[None]).sum(-1)

    w_kp = np.ascontiguousarray(
        weight.reshape(KC, P, HF).transpose(1, 0, 2).reshape(P, KC * HF)
    ).astype(BF16)
    wdst_kp = np.ascontiguousarray(
        wdst.reshape(KC, P, H).transpose(1, 0, 2).reshape(P, KC * H)
    ).astype(BF16)

    in_maps = []
    for core in range(NCORES):
        b = core // 2
        half = core % 2
        # xt layout [p, jh, kc, j']: x[b][jh*1024 + j', kc*128 + p]
        xt = x[b].T.reshape(KC, P, 2, N // 2)          # [kc, p, jh, j']
        xt_kp = np.ascontiguousarray(
            xt.transpose(1, 2, 0, 3).reshape(P, 2 * KC * (N // 2))
        ).astype(BF16)
        adjt = adj[b].T[:, half * ROWS:(half + 1) * ROWS]  # [N, ROWS]
        in_maps.append({
            "adjt": np.ascontiguousarray(adjt, dtype=np.float32).astype(FP8),
            "w": w_kp,
            "wdst": wdst_kp,
            "xt": xt_kp,
        })
    return in_maps


def _run_device(in_maps):
    from concourse import bass_utils

    nc = _get_nc()
    res = bass_utils.run_bass_kernel_spmd(
        nc, in_maps, core_ids=list(range(NCORES)))
    return [dict(r) for r in res.results]


def _run_device_subprocess(in_maps):
    """Fresh-process fallback: a wedged accelerator surfaces as
    NRT_EXEC_UNIT_UNRECOVERABLE and poisons the in-process PJRT client;
    a new process gets a fresh axon session and a reset device."""
    import os
    import pickle
    import subprocess
    import sys
    import tempfile

    d = tempfile.mkdtemp(prefix="gat_kernel_")
    inp = os.path.join(d, "in.pkl")
    outp = os.path.join(d, "out.pkl")
    with open(inp, "wb") as f:
        pickle.dump(in_maps, f)
    code = (
        "import pickle, sys\n"
        f"sys.path.insert(0, {os.path.dirname(os.path.abspath(__file__))!r})\n"
        "import kernel\n"
        f"in_maps = pickle.load(open({inp!r}, 'rb'))\n"
        f"pickle.dump(kernel._run_device(in_maps), open({outp!r}, 'wb'))\n"
    )
    env = dict(os.environ, GAT_KERNEL_SUBPROC="1")
    subprocess.run([sys.executable, "-c", code], check=True, env=env,
                   timeout=1200)
    with open(outp, "rb") as f:
        return pickle.load(f)


def kernel(x, adj, weight, attn_src, attn_dst):
    import os
    import time

    in_maps = _make_in_maps(x, adj, weight, attn_dst)
    try:
        results = _run_device(in_maps)
    except Exception:
        if os.environ.get("GAT_KERNEL_SUBPROC") == "1":
            raise
        time.sleep(2)
        results = _run_device_subprocess(in_maps)

    out = np.empty((B, N, HF), dtype=np.float32)
    for core in range(NCORES):
        b = core // 2
        half = core % 2
        for q in range(IC):
            r0 = half * ROWS + q * P
            out[b, r0:r0 + P, :] = results[core][f"out{q}"].astype(np.float32)
    return out
